# revision 29
# baseline (speedup 1.0000x reference)
"""CeNN front-end Trainium2 kernel — time-skewed schedule, PE-centric update.

Reference computation (per batch image u [1,H,W]):
    control = conv3x3_same(u, W_B)                         # [64,H,W]
    x0 = control
    x_{k+1} = alpha*x_k + beta*(conv3x3_same(tanh(x_k), WA_eff) + control
                                + bias)      (WA_eff diag center >= 1), 16x.

Distribution: 8 cores = (batch b 0..3) x (H half); each core owns a
272-row slab (256 valid + 16 halo rows), zero inter-core communication.

Schedule: T=4 steps per DRAM pass (4 passes), slab processed as NT=8
sequential 34-row tiles, TIME-SKEWED (tile n at step k updates rows
(b_{n-1}-k, b_n-k]); 2-row x_k boundaries carried tile-to-tile in SBUF,
so zero halo recompute/reload.  Pass 0 (control from u) is interleaved
into pass 1.

Per-row step = 11 accumulating quadrant matmuls (K=64,M=64,N=512):
9 conv taps + identity tap adding C = beta*(control+bias) + identity tap
adding alpha*x (alpha dithered between its two bf16 neighbours per step
to fix the bf16 quantization of alpha — validated 6.5e-3 rel).  The
"update" is then just a psum->SBUF bf16 copy: full-width aligned pairs
on ScalarE, partition-crossed halves on VectorE.  tanh runs on ScalarE
merged 3 rows per op ((N+352)/1.2ns cost model makes merging ~40%
cheaper); next-step tanh is emitted as soon as its source rows' copies
are emitted, so PE never waits a step boundary.
"""

import numpy as np
import ml_dtypes

import concourse.bacc as bacc
import concourse.tile as tile
from concourse import mybir
from concourse.bass_utils import run_bass_kernel_spmd

F32 = mybir.dt.float32
BF16 = mybir.dt.bfloat16
AF = mybir.ActivationFunctionType
ALU = mybir.AluOpType

SLAB, W, WP = 272, 512, 514
T, HS = 4, 34
NT = SLAB // HS
RH = 20
NJ_TOP = 25            # j: 0..19 main, 20 CS0, 21/22 D2, 23/24 D3
CS0 = 40
DSLOT = {2: (41, 42), 3: (43, 44)}
NPASS = 16 // T
RC0 = 8
NCHUNK0 = SLAB // RC0
UROWS = SLAB + 2


def _half(s, pi):
    if s >= 40:
        return pi
    return pi if s < RH else 1 - pi


def _j(s):
    if s >= 40:
        return s - 20
    return s if s < RH else s - RH


def _P(h):
    return slice(64 * h, 64 * h + 64)


def _ssrc(n, k, d):
    """Slot holding x_{k-1}[b0+d] when tile n runs step k (d >= -k)."""
    if n == 1:
        return d + T
    if k == 1:
        return CS0 if d == -1 else d + T
    if k == 2:
        return d + T
    if d == -k:
        return DSLOT[k - 1][0]
    if d == -(k - 1):
        return DSLOT[k - 1][1]
    return d + T


def _tile_geom(n, k):
    b0 = (n - 1) * HS
    r_lo = 0 if n == 1 else b0 - k + 1
    r_hi = SLAB - 1 if n == NT else b0 + HS - k
    return b0, r_lo, r_hi, r_lo - b0 + T, r_hi - b0 + T


def _plan_step(n, k):
    """Groups for tile n step k.  Each group is (kind, rows) where rows is
    a list of (s_out, ph, pidx) with pidx 0 -> P0 / 1 -> P1 of the group.
    kinds: g4 (aligned pair + crossed pair), g2p (aligned pair), g4s
    (4 singles on 4 quadrants), g2m (leftover singles).
    Returns (groups, written: slot -> group idx, align: s_out -> bool)."""
    pi = (n - 1) % 2
    b0, r_lo, r_hi, s_lo, s_hi = _tile_geom(n, k)
    remap_j = None
    if n > 1 and k >= 3:
        remap_j = T - k + 1
    pair_js = [j for j in range(s_lo, s_hi - RH + 1) if j != remap_j]
    top_singles = [j for j in range(max(s_lo, s_hi - RH + 1), RH)]
    if remap_j is not None and remap_j >= s_lo:
        top_singles.insert(0, remap_j)
    bot_singles = [j for j in range(0, s_hi - RH + 1) if j not in pair_js]

    groups = []
    align = {}
    i = 0
    while i + 1 < len(pair_js):
        ja, jb = pair_js[i], pair_js[i + 1]
        rows = [(ja, pi, 0), (ja + RH, 1 - pi, 0),
                (jb, 1 - pi, 1), (jb + RH, pi, 1)]
        align[ja] = True
        groups.append(("g4", ja, jb, rows))
        i += 2
    if i < len(pair_js):
        ja = pair_js[i]
        rows = [(ja, pi, 0), (ja + RH, 1 - pi, 0)]
        align[ja] = True
        groups.append(("g2p", ja, None, rows))
    ti = bi = 0
    while ti < len(top_singles) or bi < len(bot_singles):
        jts = top_singles[ti:ti + 2]
        jbs = bot_singles[bi:bi + 2]
        ti += 2
        bi += 2
        jts = jts[:1]
        jbs = jbs[:1]
        ti -= 1
        bi -= 1
        rows = []
        if len(jts) >= 1:
            rows.append((jts[0], pi, 0))
        if len(jbs) >= 1:
            rows.append((jbs[0] + RH, 1 - pi, 0))
        kind = "g2m"
        groups.append((kind, None, None, rows))

    written = {}
    for gi, g in enumerate(groups):
        for (s_out, ph, pidx) in g[3]:
            written[s_out] = gi
    return groups, written, align


def _need_tb(n, k, pi):
    need = {}
    b0, r_lo, r_hi, _, _ = _tile_geom(n, k)
    for r in range(r_lo - 1, r_hi + 2):
        s = _ssrc(n, k, r - b0)
        need.setdefault(_j(s), set()).add(_half(s, pi))
    return need


def _tanh_runs(n, k, pi):
    """Merge the step's tanh coverage into runs of <=3 consecutive j with
    identical half-sets.  Returns list of (j0, m, halves)."""
    need = _need_tb(n, k, pi)
    runs = []
    for j in sorted(need):
        hs_ = need[j]
        mcap = 2 if j >= RH else 3
        if (runs and runs[-1][2] == hs_
                and (j >= RH) == (runs[-1][0] >= RH)
                and runs[-1][0] + runs[-1][1] == j
                and runs[-1][1] < mcap):
            runs[-1] = (runs[-1][0], runs[-1][1] + 1, hs_)
        else:
            runs.append((j, 1, hs_))
    return runs


class _TilePlan:
    """Per-tile tanh planning/emission state, so the previous tile's step
    loop can emit this tile's load-dependent tanh runs early."""

    def __init__(self, nc, n, xs, thpool):
        self.nc = nc
        self.n = n
        self.xs = xs
        self.thpool = thpool
        self.pi = (n - 1) % 2
        self.plans = {k: _plan_step(n, k) for k in range(1, T + 1)}
        self.truns = {k: _tanh_runs(n, k, self.pi) for k in range(1, T + 1)}
        self.thd = {k: {} for k in range(1, T + 1)}
        self.emitted = set()
        self.run_gate = {}
        for k in range(1, T + 1):
            w_prev = self.plans[k - 1][1] if k > 1 else {}
            for (j0, m, hs_) in self.truns[k]:
                g = -1
                for jj in range(j0, j0 + m):
                    for h in hs_:
                        if jj >= RH:
                            sl = jj + 20
                        else:
                            sl = jj if h == self.pi else jj + RH
                        if sl in w_prev:
                            g = max(g, w_prev[sl])
                self.run_gate[(k, j0)] = g

    def emit_run(self, k, j0, m, hs_):
        if (k, j0) in self.emitted:
            return
        self.emitted.add((k, j0))
        nc, xs = self.nc, self.xs
        if j0 >= RH:
            tt = self.thpool.tile([128, 2, WP], BF16, name="tth",
                                  tag="hold", bufs=6)
        else:
            tt = self.thpool.tile([128, 3, WP], BF16, name="tt", bufs=20)
        for jj in range(j0, j0 + m):
            self.thd[k][jj] = (tt, jj - j0)
        if hs_ == {0, 1}:
            nc.scalar.activation(out=tt[:, 0:m, :],
                                 in_=xs[:, j0:j0 + m, :], func=AF.Tanh)
        else:
            (h,) = hs_
            nc.scalar.activation(out=tt[_P(h), 0:m, :],
                                 in_=xs[_P(h), j0:j0 + m, :], func=AF.Tanh)

    def early(self):
        # load-dependent runs: all of step 1 + step-2 runs with no
        # same-tile writer (carry-fed main slots, written by prev tile)
        for (j0, m, hs_) in self.truns[1]:
            self.emit_run(1, j0, m, hs_)
        for (j0, m, hs_) in self.truns[2]:
            if self.run_gate[(2, j0)] < 0 and j0 < RH:
                self.emit_run(2, j0, m, hs_)

    def start_rest(self):
        for k in range(1, T + 1):
            for (j0, m, hs_) in self.truns[k]:
                if self.run_gate[(k, j0)] < 0:
                    self.emit_run(k, j0, m, hs_)


def build():
    nc = bacc.Bacc("TRN2", target_bir_lowering=False, debug=False,
                   num_devices=8)

    u_in = nc.dram_tensor("u_in", [UROWS, W], BF16, kind="ExternalInput")
    wa_in = nc.dram_tensor("wa_in", [64, 12, 64], BF16, kind="ExternalInput")
    wb_in = nc.dram_tensor("wb_in", [10, 64], BF16, kind="ExternalInput")
    nbias_in = nc.dram_tensor("nbias_in", [64, 1], F32, kind="ExternalInput")
    alpha_in = nc.dram_tensor("alpha_in", [1, 1], F32, kind="ExternalInput")
    x_out = nc.dram_tensor("x_out", [64, SLAB, WP], BF16,
                           kind="ExternalOutput")

    Xd = [nc.dram_tensor(f"Xd{i}", [64, SLAB, WP], BF16, kind="Internal")
          for i in range(2)]
    Chi_d = nc.dram_tensor("Chi", [64, SLAB, WP], BF16, kind="Internal")

    with tile.TileContext(nc) as tc:
        with tc.tile_pool(name="singles", bufs=1) as singles:
            wa_t = singles.tile([128, 12, 64], BF16)
            nc.sync.dma_start(out=wa_t[0:64], in_=wa_in[:, :, :])
            nc.sync.dma_start(out=wa_t[64:128], in_=wa_in[:, :, :])
            wb_t = singles.tile([10, 64], BF16)
            nc.sync.dma_start(out=wb_t, in_=wb_in[:, :])
            nbias_t = singles.tile([64, 1], F32)
            nc.sync.dma_start(out=nbias_t, in_=nbias_in[:, :])
            alpha_t = singles.tile([128, 1], F32)
            nc.sync.dma_start(out=alpha_t,
                              in_=alpha_in[:, :].to_broadcast((128, 1)))
            beta_t = singles.tile([128, 1], F32)
            nc.vector.tensor_scalar(out=beta_t, in0=alpha_t, scalar1=-1.0,
                                    scalar2=1.0, op0=ALU.mult, op1=ALU.add)

            with tc.tile_pool(name="p0u", bufs=2) as p0u, \
                 tc.tile_pool(name="p0st", bufs=1) as p0st, \
                 tc.tile_pool(name="xs", bufs=2) as xpool, \
                 tc.tile_pool(name="chs", bufs=2) as chpool, \
                 tc.tile_pool(name="th", bufs=16) as thpool, \
                 tc.tile_pool(name="ps", bufs=3, space="PSUM") as pspool:

                u9bufs = [p0u.tile([10, RC0, W], BF16, tag="u9",
                                   name="u9a"),
                          p0u.tile([10, RC0, W], BF16, tag="u9",
                                   name="u9b")]
                for _u9 in u9bufs:
                    nc.vector.memset(_u9[0:1, :, :], 1.0)

                def emit_chunk(c):
                    c0 = RC0 * c
                    u9 = u9bufs[c % 2]
                    # zero edge cols (kw=0 misses col 0, kw=2 misses col
                    # 511), then restore the ones-row edges
                    nc.vector.memset(u9[0:10, 0:RC0, 0:1], 0.0)
                    nc.vector.memset(u9[0:10, 0:RC0, W - 1:W], 0.0)
                    nc.vector.memset(u9[0:1, 0:RC0, 0:1], 1.0)
                    nc.vector.memset(u9[0:1, 0:RC0, W - 1:W], 1.0)
                    for t9 in range(9):
                        kh, kw = divmod(t9, 3)
                        c_lo = max(0, 1 - kw)
                        c_hi = min(W, W + 1 - kw)
                        nc.sync.dma_start(
                            out=u9[t9 + 1:t9 + 2, 0:RC0, c_lo:c_hi],
                            in_=u_in[c0 + kh:c0 + kh + RC0,
                                     c_lo + kw - 1:c_hi + kw - 1],
                        )
                    xst = p0st.tile([64, RC0, WP], BF16, tag="xst",
                                    name="xst")
                    chst = p0st.tile([64, RC0, WP], BF16, tag="chst",
                                     name="chst")
                    for st in (xst, chst):
                        nc.vector.memset(st[:, :, 0:1], 0.0)
                        nc.vector.memset(st[:, :, 513:514], 0.0)
                    for t in range(RC0):
                        pc = pspool.tile([64, 512], F32, tag="pc", bufs=1,
                                         name="pc")
                        nc.tensor.matmul(pc, wb_t[0:10, :], u9[0:10, t, :],
                                         start=True, stop=True)
                        nc.scalar.activation(out=xst[:, t, 1:513], in_=pc,
                                             func=AF.Identity,
                                             bias=nbias_t[0:64], scale=1.0)
                        nc.vector.tensor_scalar(
                            out=chst[:, t, 1:513], in0=pc,
                            scalar1=beta_t[0:64], scalar2=None, op0=ALU.mult)
                    nc.sync.dma_start(out=Xd[0][:, c0:c0 + RC0, :], in_=xst)
                    nc.sync.dma_start(out=Chi_d[:, c0:c0 + RC0, :], in_=chst)

                chunks_done = 0
                tiles = [(p, n) for p in range(1, NPASS + 1)
                         for n in range(1, NT + 1)]
                bufs = {}

                tps = {}

                def alloc(i):
                    bufs[i] = (xpool.tile([128, NJ_TOP, WP], BF16,
                                          tag="xs", name="xs"),
                               chpool.tile([128, RH, WP], BF16, tag="ch",
                                           name="ch"))

                def loads(i):
                    pp, nn = tiles[i]
                    xsb, chb = bufs[i]
                    _emit_loads(nc, nn, xsb, chb, Xd[(pp - 1) % 2], Chi_d)

                alloc(0)
                for i, (p, n) in enumerate(tiles):
                    if p == 1:
                        need_c = min(NCHUNK0, -(-(n * HS + 36) // RC0))
                        if n == NT:
                            need_c = NCHUNK0
                        while chunks_done < need_c:
                            emit_chunk(chunks_done)
                            chunks_done += 1
                    if i == 0:
                        loads(0)
                        tps[0] = _TilePlan(nc, n, bufs[0][0], thpool)
                        tps[0].early()
                    if i + 1 < len(tiles):
                        alloc(i + 1)
                        loads(i + 1)
                        tps[i + 1] = _TilePlan(nc, tiles[i + 1][1],
                                               bufs[i + 1][0], thpool)
                    xs_cur, ch_cur = bufs.pop(i)
                    xs_nxt = bufs[i + 1][0] if (n < NT and i + 1 in bufs) \
                        else None
                    tp = tps.pop(i)
                    tp.start_rest()
                    early_fn = tps[i + 1].early if i + 1 in tps else None
                    _emit_tile(nc, p, n, tp, xs_cur, xs_nxt, ch_cur,
                               Xd[(p - 1) % 2], Xd[p % 2], Chi_d, x_out,
                               wa_t, thpool, pspool, p == NPASS, early_fn)

    nc.compile()
    return nc


def _emit_loads(nc, n, xs, ch, src_d, Chi_d):
    pi = (n - 1) % 2
    TOP, BOT = _P(pi), _P(1 - pi)
    b0 = (n - 1) * HS
    if n == 1:
        nc.vector.memset(xs[TOP, 2:4, :], 0.0)
        nc.vector.memset(xs[TOP, 20:21, :], 0.0)
    else:
        nc.sync.dma_start(out=xs[TOP, 20:21, :],
                          in_=src_d[:, b0 - 1:b0, :])
        nc.vector.memset(xs[TOP, 1:2, :], 0.0)
    nc.sync.dma_start(out=xs[TOP, 4:20, :], in_=src_d[:, b0:b0 + 16, :])
    if n == NT:
        nc.sync.dma_start(out=xs[BOT, 0:18, :],
                          in_=src_d[:, b0 + 16:b0 + 34, :])
        nc.vector.memset(xs[BOT, 18:19, :], 0.0)
    else:
        nc.sync.dma_start(out=xs[BOT, 0:19, :],
                          in_=src_d[:, b0 + 16:b0 + 35, :])
    if n == 1:
        nc.gpsimd.dma_start(out=ch[TOP, 4:20, :], in_=Chi_d[:, 0:16, :])
    else:
        nc.gpsimd.dma_start(out=ch[TOP, 1:20, :],
                            in_=Chi_d[:, b0 - 3:b0 + 16, :])
    nc.gpsimd.dma_start(out=ch[BOT, 0:18, :],
                        in_=Chi_d[:, b0 + 16:b0 + 34, :])


def _emit_tile(nc, p, n, tp, xs, xs_nxt, ch, src_d, dst_d, Chi_d, x_out,
               wa_t, thpool, pspool, last, early_fn):
    pi = (n - 1) % 2
    TOP, BOT = _P(pi), _P(1 - pi)
    b0 = (n - 1) * HS
    plans = tp.plans
    truns = tp.truns
    thd = tp.thd
    run_gate = tp.run_gate
    emit_tanh_run = tp.emit_run

    # ---------------- steps ----------------
    for k in range(1, T + 1):
        groups, written, align = plans[k]
        th = thd[k]
        atap = 10 + (((p - 1) * T + (k - 1)) % 2)

        def ssrc(d):
            return _ssrc(n, k, d)

        def row_taps(s_out, ph, ps_tile, pf_tile):
            d = s_out - T
            dhalf = _half(s_out, pi)
            dj = _j(s_out)
            out_ps = ps_tile[ph * 64:ph * 64 + 64, :]
            main, foreign = [], []
            for t9 in range(9):
                kh, kw = divmod(t9, 3)
                ss = ssrc(d + kh - 1)
                shalf, sj = _half(ss, pi), _j(ss)
                ent = (t9, shalf, sj, kw)
                (main if shalf == dhalf else foreign).append(ent)
            ops = []
            for idx, (t9, shalf, sj, kw) in enumerate(main):
                tt, off = th[sj]
                ops.append(dict(
                    out=out_ps, lhsT=wa_t[_P(shalf), t9, :],
                    rhs=tt[_P(shalf), off, kw:kw + 512],
                    start=(idx == 0), stop=False,
                    tile_position=(shalf * 64, ph * 64)))
            # alpha tap: rhs is raw x_{k-1} of this row (dithered weight)
            sin = ssrc(d)
            ops.append(dict(
                out=out_ps, lhsT=wa_t[_P(dhalf), atap, :],
                rhs=xs[_P(dhalf), _j(sin), 1:513],
                start=False, stop=True,
                tile_position=(dhalf * 64, ph * 64)))
            if foreign:
                out_pf = pf_tile[ph * 64:ph * 64 + 64, :]
                for idx, (t9, shalf, sj, kw) in enumerate(foreign):
                    tt, off = th[sj]
                    ops.append(dict(
                        out=out_pf, lhsT=wa_t[_P(shalf), t9, :],
                        rhs=tt[_P(shalf), off, kw:kw + 512],
                        start=(idx == 0), stop=(idx == len(foreign) - 1),
                        tile_position=(shalf * 64, ph * 64)))
            return ops, bool(foreign)

        def fadd(s_out, ph, pf_tile):
            dhalf = _half(s_out, pi)
            dj = _j(s_out)
            nc.vector.scalar_tensor_tensor(
                out=xs[_P(dhalf), dj, 1:513],
                in0=xs[_P(dhalf), dj, 1:513],
                scalar=1.0, in1=pf_tile[_P(ph), :],
                op0=ALU.bypass, op1=ALU.add)

        def need_foreign(s_out):
            d = s_out - T
            dhalf = _half(s_out, pi)
            for kh in (0, 2):
                if _half(ssrc(d + kh - 1), pi) != dhalf:
                    return True
            return False

        carry_gi = -1
        if k < T and xs_nxt is not None:
            carry_gi = max(written[HS + T - k - 1], written[HS + T - k])

        for gi, g in enumerate(groups):
            kind, ja, jb, rows = g
            P0 = pspool.tile([128, 512], F32, tag="P0", bufs=4, name="P0")
            P1 = None
            if len(rows) > 2:
                P1 = pspool.tile([128, 512], F32, tag="P1", bufs=2,
                                 name="P1")
            PF = None
            if any(need_foreign(s) for (s, _, _) in rows):
                PF = pspool.tile([128, 512], F32, tag="PF", bufs=1,
                                 name="PF")
            seqs = []
            folds = []
            for (s_out, ph, pidx) in rows:
                Pt = P0 if pidx == 0 else P1
                ops, f = row_taps(s_out, ph, Pt, PF)
                seqs.append(ops)
                if f:
                    folds.append((s_out, ph))
            nmax = max((len(s) for s in seqs), default=0)
            for t in range(nmax):
                for s in seqs:
                    if t < len(s):
                        nc.tensor.matmul(
                            s[t]["out"], s[t]["lhsT"], s[t]["rhs"],
                            start=s[t]["start"], stop=s[t]["stop"],
                            skip_group_check=True,
                            tile_position=s[t]["tile_position"])
            # ---- copies: psum -> xs (the whole update) ----
            if kind in ("g4", "g2p"):
                # x' = psum + C, full width (P0 halves align with xs)
                nc.vector.tensor_add(out=xs[:, ja, 1:513], in0=P0[:, :],
                                     in1=ch[:, ja, 1:513])
                if kind == "g4":
                    nc.vector.tensor_add(out=xs[TOP, jb, 1:513],
                                         in0=P1[BOT, :],
                                         in1=ch[TOP, jb, 1:513])
                    nc.vector.tensor_add(out=xs[BOT, jb, 1:513],
                                         in0=P1[TOP, :],
                                         in1=ch[BOT, jb, 1:513])
            else:
                for (s_out, ph, pidx) in rows:
                    Pt = P0 if pidx == 0 else P1
                    dhalf = _half(s_out, pi)
                    dj = _j(s_out)
                    nc.vector.tensor_add(out=xs[_P(dhalf), dj, 1:513],
                                         in0=Pt[_P(ph), :],
                                         in1=ch[_P(dhalf), dj, 1:513])
            for (s_out, ph) in folds:
                fadd(s_out, ph, PF)
            if gi == carry_gi:
                s_src = HS + T - k - 1
                j_src = s_src - RH
                dj0 = 2 if k == 1 else _j(DSLOT[k][0])
                nc.vector.tensor_copy(out=xs_nxt[BOT, dj0:dj0 + 2, :],
                                      in_=xs[BOT, j_src:j_src + 2, :])
            if k < T:
                for (j0, m, hs_) in truns[k + 1]:
                    if run_gate[(k + 1, j0)] == gi:
                        emit_tanh_run(k + 1, j0, m, hs_)
        if k == 1 and early_fn is not None:
            # emit the next tile's load-dependent tanh now so it overlaps
            # the rest of this tile instead of stalling at the boundary
            early_fn()

    # ---------------- store x_T ----------------
    s_lo_st = 4 if n == 1 else 1
    s_hi_st = (SLAB - 1 - b0 + T) if n == NT else HS
    r_top0 = b0 - T + s_lo_st
    n_top = RH - s_lo_st
    r_bot0 = b0 - T + RH
    n_bot = s_hi_st - RH + 1
    dst = x_out if last else dst_d
    nc.sync.dma_start(out=dst[:, r_top0:r_top0 + n_top, :],
                      in_=xs[_P(pi), s_lo_st:RH, :])
    nc.sync.dma_start(out=dst[:, r_bot0:r_bot0 + n_bot, :],
                      in_=xs[_P(1 - pi), 0:n_bot, :])


def host_prep(u, W_B, W_A, bias, alpha_logit):
    alpha = np.float32(1.0 / (1.0 + np.exp(-np.float64(alpha_logit))))
    beta = np.float32(1.0) - alpha

    WAe = np.array(W_A, dtype=np.float32).copy()
    idx = np.arange(64)
    WAe[idx, idx, 1, 1] = np.maximum(WAe[idx, idx, 1, 1], np.float32(1.0))

    # alpha dither: the two bf16 neighbours of alpha
    cands = np.unique(np.array(
        [ml_dtypes.bfloat16(alpha * (1 + eps))
         for eps in np.linspace(-0.02, 0.02, 2001)],
        dtype=ml_dtypes.bfloat16).astype(np.float32))
    lo_c = cands[cands <= alpha]
    hi_c = cands[cands >= alpha]
    a_lo = lo_c[-1] if len(lo_c) else np.float32(a_bf)
    a_hi = hi_c[0] if len(hi_c) else np.float32(a_bf)

    wa_taps = np.zeros((64, 12, 64), dtype=np.float32)
    for t9 in range(9):
        kh, kw = divmod(t9, 3)
        wa_taps[:, t9, :] = (beta * WAe[:, :, kh, kw]).T   # [cin, cout]
    eye = np.eye(64, dtype=np.float32)
    wa_taps[:, 9, :] = eye
    wa_taps[:, 10, :] = a_lo * eye
    wa_taps[:, 11, :] = a_hi * eye
    wa_taps = wa_taps.astype(ml_dtypes.bfloat16)

    bias_vec = np.array(bias, dtype=np.float32).reshape(64)
    wb10 = np.zeros((10, 64), dtype=np.float32)
    wb10[0, :] = bias_vec
    for t9 in range(9):
        kh, kw = divmod(t9, 3)
        wb10[t9 + 1, :] = W_B[:, 0, kh, kw]
    wb10 = wb10.astype(ml_dtypes.bfloat16)
    nbias = (-bias_vec).reshape(64, 1).astype(np.float32)
    alpha_arr = np.full((1, 1), alpha, dtype=np.float32)

    H = u.shape[2]
    in_maps = []
    for core in range(8):
        b, h = divmod(core, 2)
        img = np.asarray(u[b, 0], dtype=np.float32)        # [H, 512]
        u_slab = np.zeros((UROWS, W), dtype=np.float32)
        if h == 0:
            u_slab[1:UROWS] = img[0:SLAB + 1]
        else:
            off = H - SLAB
            u_slab[0:UROWS - 1] = img[off - 1:H]
        in_maps.append({
            "u_in": u_slab.astype(ml_dtypes.bfloat16),
            "wa_in": wa_taps,
            "wb_in": wb10,
            "nbias_in": nbias,
            "alpha_in": alpha_arr,
        })
    return in_maps


_NC_CACHE = {}


def _get_nc():
    if "nc" not in _NC_CACHE:
        _NC_CACHE["nc"] = build()
    return _NC_CACHE["nc"]


def kernel(u, W_B, W_A, bias, alpha_logit, _trace=False):
    u = np.asarray(u, dtype=np.float32)
    B, _, H, Wc = u.shape
    nc = _get_nc()
    in_maps = host_prep(u, W_B, W_A, bias, alpha_logit)
    res = run_bass_kernel_spmd(nc, in_maps, core_ids=list(range(8)),
                               trace=_trace)
    VALID = H // 2
    out = np.zeros((B, 64, H, Wc), dtype=np.float32)
    for core in range(8):
        b, h = divmod(core, 2)
        xo = np.asarray(res.results[core]["x_out"])[:, :, 1:513]
        xo = xo.astype(np.float32)
        if h == 0:
            out[b, :, 0:VALID, :] = xo[:, 0:VALID, :]
        else:
            out[b, :, VALID:H, :] = xo[:, SLAB - VALID:SLAB, :]
    kernel._last_results = res
    return out


# revision 30
# speedup vs baseline: 1.0735x; 1.0735x over previous
"""CeNN front-end Trainium2 kernel — time-skewed schedule, PE-centric update.

Reference computation (per batch image u [1,H,W]):
    control = conv3x3_same(u, W_B)                         # [64,H,W]
    x0 = control
    x_{k+1} = alpha*x_k + beta*(conv3x3_same(tanh(x_k), WA_eff) + control
                                + bias)      (WA_eff diag center >= 1), 16x.

Distribution: 8 cores = (batch b 0..3) x (H half); each core owns a
272-row slab (256 valid + 16 halo rows), zero inter-core communication.

Schedule: T=4 steps per DRAM pass (4 passes), slab processed as NT=8
sequential 34-row tiles, TIME-SKEWED (tile n at step k updates rows
(b_{n-1}-k, b_n-k]); 2-row x_k boundaries carried tile-to-tile in SBUF,
so zero halo recompute/reload.  Pass 0 (control from u) is interleaved
into pass 1.

Per-row step = 11 accumulating quadrant matmuls (K=64,M=64,N=512):
9 conv taps + identity tap adding C = beta*(control+bias) + identity tap
adding alpha*x (alpha dithered between its two bf16 neighbours per step
to fix the bf16 quantization of alpha — validated 6.5e-3 rel).  The
"update" is then just a psum->SBUF bf16 copy: full-width aligned pairs
on ScalarE, partition-crossed halves on VectorE.  tanh runs on ScalarE
merged 3 rows per op ((N+352)/1.2ns cost model makes merging ~40%
cheaper); next-step tanh is emitted as soon as its source rows' copies
are emitted, so PE never waits a step boundary.
"""

import numpy as np
import ml_dtypes

import concourse.bacc as bacc
import concourse.tile as tile
from concourse import mybir
from concourse.bass_utils import run_bass_kernel_spmd

F32 = mybir.dt.float32
BF16 = mybir.dt.bfloat16
AF = mybir.ActivationFunctionType
ALU = mybir.AluOpType

SLAB, W, WP = 272, 512, 514
T, HS = 4, 34
NT = SLAB // HS
RH = 20
NJ_TOP = 25            # j: 0..19 main, 20 CS0, 21/22 D2, 23/24 D3
CS0 = 40
DSLOT = {2: (41, 42), 3: (43, 44)}
NPASS = 16 // T
RC0 = 8
NCHUNK0 = SLAB // RC0
UROWS = SLAB + 2


def _half(s, pi):
    if s >= 40:
        return pi
    return pi if s < RH else 1 - pi


def _j(s):
    if s >= 40:
        return s - 20
    return s if s < RH else s - RH


def _P(h):
    return slice(64 * h, 64 * h + 64)


def _ssrc(n, k, d):
    """Slot holding x_{k-1}[b0+d] when tile n runs step k (d >= -k)."""
    if n == 1:
        return d + T
    if k == 1:
        return CS0 if d == -1 else d + T
    if k == 2:
        return d + T
    if d == -k:
        return DSLOT[k - 1][0]
    if d == -(k - 1):
        return DSLOT[k - 1][1]
    return d + T


def _tile_geom(n, k):
    b0 = (n - 1) * HS
    r_lo = 0 if n == 1 else b0 - k + 1
    r_hi = SLAB - 1 if n == NT else b0 + HS - k
    return b0, r_lo, r_hi, r_lo - b0 + T, r_hi - b0 + T


def _plan_step(n, k):
    """Groups for tile n step k.  Each group is (kind, rows) where rows is
    a list of (s_out, ph, pidx) with pidx 0 -> P0 / 1 -> P1 of the group.
    kinds: g4 (aligned pair + crossed pair), g2p (aligned pair), g4s
    (4 singles on 4 quadrants), g2m (leftover singles).
    Returns (groups, written: slot -> group idx, align: s_out -> bool)."""
    pi = (n - 1) % 2
    b0, r_lo, r_hi, s_lo, s_hi = _tile_geom(n, k)
    remap_j = None
    if n > 1 and k >= 3:
        remap_j = T - k + 1
    pair_js = [j for j in range(s_lo, s_hi - RH + 1) if j != remap_j]
    top_singles = [j for j in range(max(s_lo, s_hi - RH + 1), RH)]
    if remap_j is not None and remap_j >= s_lo:
        top_singles.insert(0, remap_j)
    bot_singles = [j for j in range(0, s_hi - RH + 1) if j not in pair_js]

    groups = []
    align = {}
    i = 0
    while i + 1 < len(pair_js):
        ja, jb = pair_js[i], pair_js[i + 1]
        rows = [(ja, pi, 0), (ja + RH, 1 - pi, 0),
                (jb, 1 - pi, 1), (jb + RH, pi, 1)]
        align[ja] = True
        groups.append(("g4", ja, jb, rows))
        i += 2
    if i < len(pair_js):
        ja = pair_js[i]
        rows = [(ja, pi, 0), (ja + RH, 1 - pi, 0)]
        align[ja] = True
        groups.append(("g2p", ja, None, rows))
    ti = bi = 0
    while ti < len(top_singles) or bi < len(bot_singles):
        jts = top_singles[ti:ti + 2]
        jbs = bot_singles[bi:bi + 2]
        ti += 2
        bi += 2
        jts = jts[:1]
        jbs = jbs[:1]
        ti -= 1
        bi -= 1
        rows = []
        if len(jts) >= 1:
            rows.append((jts[0], pi, 0))
        if len(jbs) >= 1:
            rows.append((jbs[0] + RH, 1 - pi, 0))
        kind = "g2m"
        groups.append((kind, None, None, rows))

    written = {}
    for gi, g in enumerate(groups):
        for (s_out, ph, pidx) in g[3]:
            written[s_out] = gi
    return groups, written, align


def _need_tb(n, k, pi):
    need = {}
    b0, r_lo, r_hi, _, _ = _tile_geom(n, k)
    for r in range(r_lo - 1, r_hi + 2):
        s = _ssrc(n, k, r - b0)
        need.setdefault(_j(s), set()).add(_half(s, pi))
    return need


def _tanh_runs(n, k, pi):
    """Merge the step's tanh coverage into runs of <=3 consecutive j with
    identical half-sets.  Returns list of (j0, m, halves)."""
    need = _need_tb(n, k, pi)
    runs = []
    for j in sorted(need):
        hs_ = need[j]
        mcap = 2 if j >= RH else 3
        if (runs and runs[-1][2] == hs_
                and (j >= RH) == (runs[-1][0] >= RH)
                and runs[-1][0] + runs[-1][1] == j
                and runs[-1][1] < mcap):
            runs[-1] = (runs[-1][0], runs[-1][1] + 1, hs_)
        else:
            runs.append((j, 1, hs_))
    return runs


class _TilePlan:
    """Per-tile tanh planning/emission state, so the previous tile's step
    loop can emit this tile's load-dependent tanh runs early."""

    def __init__(self, nc, n, xs, thpool):
        self.nc = nc
        self.n = n
        self.xs = xs
        self.thpool = thpool
        self.pi = (n - 1) % 2
        self.plans = {k: _plan_step(n, k) for k in range(1, T + 1)}
        self.truns = {k: _tanh_runs(n, k, self.pi) for k in range(1, T + 1)}
        self.thd = {k: {} for k in range(1, T + 1)}
        self.emitted = set()
        self.run_gate = {}
        for k in range(1, T + 1):
            w_prev = self.plans[k - 1][1] if k > 1 else {}
            for (j0, m, hs_) in self.truns[k]:
                g = -1
                for jj in range(j0, j0 + m):
                    for h in hs_:
                        if jj >= RH:
                            sl = jj + 20
                        else:
                            sl = jj if h == self.pi else jj + RH
                        if sl in w_prev:
                            g = max(g, w_prev[sl])
                self.run_gate[(k, j0)] = g

    def emit_run(self, k, j0, m, hs_):
        if (k, j0) in self.emitted:
            return
        self.emitted.add((k, j0))
        nc, xs = self.nc, self.xs
        if j0 >= RH:
            tt = self.thpool.tile([128, 2, WP], BF16, name="tth",
                                  tag="hold", bufs=6)
        else:
            tt = self.thpool.tile([128, 3, WP], BF16, name="tt", bufs=20)
        for jj in range(j0, j0 + m):
            self.thd[k][jj] = (tt, jj - j0)
        if hs_ == {0, 1}:
            nc.scalar.activation(out=tt[:, 0:m, :],
                                 in_=xs[:, j0:j0 + m, :], func=AF.Tanh)
        else:
            (h,) = hs_
            nc.scalar.activation(out=tt[_P(h), 0:m, :],
                                 in_=xs[_P(h), j0:j0 + m, :], func=AF.Tanh)

    def early(self):
        # load-dependent runs: all of step 1 + step-2 runs with no
        # same-tile writer (carry-fed main slots, written by prev tile)
        for (j0, m, hs_) in self.truns[1]:
            self.emit_run(1, j0, m, hs_)
        for (j0, m, hs_) in self.truns[2]:
            if self.run_gate[(2, j0)] < 0 and j0 < RH:
                self.emit_run(2, j0, m, hs_)

    def start_rest(self):
        for k in range(1, T + 1):
            for (j0, m, hs_) in self.truns[k]:
                if self.run_gate[(k, j0)] < 0:
                    self.emit_run(k, j0, m, hs_)


def build():
    nc = bacc.Bacc("TRN2", target_bir_lowering=False, debug=False,
                   num_devices=8)

    u_in = nc.dram_tensor("u_in", [UROWS, W], BF16, kind="ExternalInput")
    wa_in = nc.dram_tensor("wa_in", [64, 12, 64], BF16, kind="ExternalInput")
    wb_in = nc.dram_tensor("wb_in", [10, 64], BF16, kind="ExternalInput")
    nbias_in = nc.dram_tensor("nbias_in", [64, 1], F32, kind="ExternalInput")
    alpha_in = nc.dram_tensor("alpha_in", [1, 1], F32, kind="ExternalInput")
    x_out = nc.dram_tensor("x_out", [64, SLAB, WP], BF16,
                           kind="ExternalOutput")

    Xd = [nc.dram_tensor(f"Xd{i}", [64, SLAB, WP], BF16, kind="Internal")
          for i in range(2)]
    Chi_d = nc.dram_tensor("Chi", [64, SLAB, WP], BF16, kind="Internal")

    with tile.TileContext(nc) as tc:
        with tc.tile_pool(name="singles", bufs=1) as singles:
            wa_t = singles.tile([128, 12, 64], BF16)
            nc.sync.dma_start(out=wa_t[0:64], in_=wa_in[:, :, :])
            nc.sync.dma_start(out=wa_t[64:128], in_=wa_in[:, :, :])
            wb_t = singles.tile([10, 64], BF16)
            nc.sync.dma_start(out=wb_t, in_=wb_in[:, :])
            nbias_t = singles.tile([64, 1], F32)
            nc.sync.dma_start(out=nbias_t, in_=nbias_in[:, :])
            alpha_t = singles.tile([128, 1], F32)
            nc.sync.dma_start(out=alpha_t,
                              in_=alpha_in[:, :].to_broadcast((128, 1)))
            beta_t = singles.tile([128, 1], F32)
            nc.vector.tensor_scalar(out=beta_t, in0=alpha_t, scalar1=-1.0,
                                    scalar2=1.0, op0=ALU.mult, op1=ALU.add)

            with tc.tile_pool(name="p0u", bufs=2) as p0u, \
                 tc.tile_pool(name="p0st", bufs=1) as p0st, \
                 tc.tile_pool(name="xs", bufs=2) as xpool, \
                 tc.tile_pool(name="chs", bufs=2) as chpool, \
                 tc.tile_pool(name="th", bufs=16) as thpool, \
                 tc.tile_pool(name="ps", bufs=3, space="PSUM") as pspool:

                u9bufs = [p0u.tile([10, RC0, W], BF16, tag="u9",
                                   name="u9a"),
                          p0u.tile([10, RC0, W], BF16, tag="u9",
                                   name="u9b")]
                for _u9 in u9bufs:
                    nc.vector.memset(_u9[0:1, :, :], 1.0)

                def emit_chunk(c):
                    c0 = RC0 * c
                    u9 = u9bufs[c % 2]
                    # zero edge cols (kw=0 misses col 0, kw=2 misses col
                    # 511), then restore the ones-row edges
                    nc.vector.memset(u9[0:10, 0:RC0, 0:1], 0.0)
                    nc.vector.memset(u9[0:10, 0:RC0, W - 1:W], 0.0)
                    nc.vector.memset(u9[0:1, 0:RC0, 0:1], 1.0)
                    nc.vector.memset(u9[0:1, 0:RC0, W - 1:W], 1.0)
                    for t9 in range(9):
                        kh, kw = divmod(t9, 3)
                        c_lo = max(0, 1 - kw)
                        c_hi = min(W, W + 1 - kw)
                        nc.sync.dma_start(
                            out=u9[t9 + 1:t9 + 2, 0:RC0, c_lo:c_hi],
                            in_=u_in[c0 + kh:c0 + kh + RC0,
                                     c_lo + kw - 1:c_hi + kw - 1],
                        )
                    xst = p0st.tile([64, RC0, WP], BF16, tag="xst",
                                    name="xst")
                    chst = p0st.tile([64, RC0, WP], BF16, tag="chst",
                                     name="chst")
                    for st in (xst, chst):
                        nc.vector.memset(st[:, :, 0:1], 0.0)
                        nc.vector.memset(st[:, :, 513:514], 0.0)
                    for t in range(RC0):
                        pc = pspool.tile([64, 512], F32, tag="pc", bufs=2,
                                         name="pc")
                        nc.tensor.matmul(pc, wb_t[0:10, :], u9[0:10, t, :],
                                         start=True, stop=True)
                        nc.scalar.activation(out=xst[:, t, 1:513], in_=pc,
                                             func=AF.Identity,
                                             bias=nbias_t[0:64], scale=1.0)
                        nc.vector.tensor_scalar(
                            out=chst[:, t, 1:513], in0=pc,
                            scalar1=beta_t[0:64], scalar2=None, op0=ALU.mult)
                    nc.sync.dma_start(out=Xd[0][:, c0:c0 + RC0, :], in_=xst)
                    nc.sync.dma_start(out=Chi_d[:, c0:c0 + RC0, :], in_=chst)

                chunks_done = 0
                tiles = [(p, n) for p in range(1, NPASS + 1)
                         for n in range(1, NT + 1)]
                bufs = {}

                tps = {}

                def alloc(i):
                    bufs[i] = (xpool.tile([128, NJ_TOP, WP], BF16,
                                          tag="xs", name="xs"),
                               chpool.tile([128, RH, WP], BF16, tag="ch",
                                           name="ch"))

                def loads(i):
                    pp, nn = tiles[i]
                    xsb, chb = bufs[i]
                    _emit_loads(nc, nn, xsb, chb, Xd[(pp - 1) % 2], Chi_d)

                alloc(0)
                for i, (p, n) in enumerate(tiles):
                    if p == 1:
                        need_c = min(NCHUNK0, -(-(n * HS + 36) // RC0))
                        if n == NT:
                            need_c = NCHUNK0
                        while chunks_done < need_c:
                            emit_chunk(chunks_done)
                            chunks_done += 1
                    if i == 0:
                        loads(0)
                        tps[0] = _TilePlan(nc, n, bufs[0][0], thpool)
                        tps[0].early()
                    if i + 1 < len(tiles):
                        alloc(i + 1)
                        loads(i + 1)
                        tps[i + 1] = _TilePlan(nc, tiles[i + 1][1],
                                               bufs[i + 1][0], thpool)
                    xs_cur, ch_cur = bufs.pop(i)
                    xs_nxt = bufs[i + 1][0] if (n < NT and i + 1 in bufs) \
                        else None
                    tp = tps.pop(i)
                    tp.start_rest()
                    early_fn = tps[i + 1].early if i + 1 in tps else None
                    _emit_tile(nc, p, n, tp, xs_cur, xs_nxt, ch_cur,
                               Xd[(p - 1) % 2], Xd[p % 2], Chi_d, x_out,
                               wa_t, thpool, pspool, p == NPASS, early_fn)

    nc.compile()
    return nc


def _emit_loads(nc, n, xs, ch, src_d, Chi_d):
    pi = (n - 1) % 2
    TOP, BOT = _P(pi), _P(1 - pi)
    b0 = (n - 1) * HS
    if n == 1:
        nc.vector.memset(xs[TOP, 2:4, :], 0.0)
        nc.vector.memset(xs[TOP, 20:21, :], 0.0)
    else:
        nc.sync.dma_start(out=xs[TOP, 20:21, :],
                          in_=src_d[:, b0 - 1:b0, :])
        nc.vector.memset(xs[TOP, 1:2, :], 0.0)
    nc.sync.dma_start(out=xs[TOP, 4:20, :], in_=src_d[:, b0:b0 + 16, :])
    if n == NT:
        nc.sync.dma_start(out=xs[BOT, 0:18, :],
                          in_=src_d[:, b0 + 16:b0 + 34, :])
        nc.vector.memset(xs[BOT, 18:19, :], 0.0)
    else:
        nc.sync.dma_start(out=xs[BOT, 0:19, :],
                          in_=src_d[:, b0 + 16:b0 + 35, :])
    if n == 1:
        nc.gpsimd.dma_start(out=ch[TOP, 4:20, :], in_=Chi_d[:, 0:16, :])
    else:
        nc.gpsimd.dma_start(out=ch[TOP, 1:20, :],
                            in_=Chi_d[:, b0 - 3:b0 + 16, :])
    nc.gpsimd.dma_start(out=ch[BOT, 0:18, :],
                        in_=Chi_d[:, b0 + 16:b0 + 34, :])


def _emit_tile(nc, p, n, tp, xs, xs_nxt, ch, src_d, dst_d, Chi_d, x_out,
               wa_t, thpool, pspool, last, early_fn):
    pi = (n - 1) % 2
    TOP, BOT = _P(pi), _P(1 - pi)
    b0 = (n - 1) * HS
    plans = tp.plans
    truns = tp.truns
    thd = tp.thd
    run_gate = tp.run_gate
    emit_tanh_run = tp.emit_run

    # ---------------- steps ----------------
    for k in range(1, T + 1):
        groups, written, align = plans[k]
        th = thd[k]
        atap = 10 + (((p - 1) * T + (k - 1)) % 2)

        def ssrc(d):
            return _ssrc(n, k, d)

        def row_taps(s_out, ph, ps_tile, pf_tile):
            d = s_out - T
            dhalf = _half(s_out, pi)
            dj = _j(s_out)
            out_ps = ps_tile[ph * 64:ph * 64 + 64, :]
            main, foreign = [], []
            for t9 in range(9):
                kh, kw = divmod(t9, 3)
                ss = ssrc(d + kh - 1)
                shalf, sj = _half(ss, pi), _j(ss)
                ent = (t9, shalf, sj, kw)
                (main if shalf == dhalf else foreign).append(ent)
            ops = []
            for idx, (t9, shalf, sj, kw) in enumerate(main):
                tt, off = th[sj]
                ops.append(dict(
                    out=out_ps, lhsT=wa_t[_P(shalf), t9, :],
                    rhs=tt[_P(shalf), off, kw:kw + 512],
                    start=(idx == 0), stop=False,
                    tile_position=(shalf * 64, ph * 64)))
            # alpha tap: rhs is raw x_{k-1} of this row (dithered weight)
            sin = ssrc(d)
            ops.append(dict(
                out=out_ps, lhsT=wa_t[_P(dhalf), atap, :],
                rhs=xs[_P(dhalf), _j(sin), 1:513],
                start=False, stop=True,
                tile_position=(dhalf * 64, ph * 64)))
            if foreign:
                out_pf = pf_tile[ph * 64:ph * 64 + 64, :]
                for idx, (t9, shalf, sj, kw) in enumerate(foreign):
                    tt, off = th[sj]
                    ops.append(dict(
                        out=out_pf, lhsT=wa_t[_P(shalf), t9, :],
                        rhs=tt[_P(shalf), off, kw:kw + 512],
                        start=(idx == 0), stop=(idx == len(foreign) - 1),
                        tile_position=(shalf * 64, ph * 64)))
            return ops, bool(foreign)

        def fadd(s_out, ph, pf_tile):
            dhalf = _half(s_out, pi)
            dj = _j(s_out)
            nc.vector.scalar_tensor_tensor(
                out=xs[_P(dhalf), dj, 1:513],
                in0=xs[_P(dhalf), dj, 1:513],
                scalar=1.0, in1=pf_tile[_P(ph), :],
                op0=ALU.bypass, op1=ALU.add)

        def need_foreign(s_out):
            d = s_out - T
            dhalf = _half(s_out, pi)
            for kh in (0, 2):
                if _half(ssrc(d + kh - 1), pi) != dhalf:
                    return True
            return False

        carry_gi = -1
        if k < T and xs_nxt is not None:
            carry_gi = max(written[HS + T - k - 1], written[HS + T - k])

        for gi, g in enumerate(groups):
            kind, ja, jb, rows = g
            P0 = pspool.tile([128, 512], F32, tag="P0", bufs=4, name="P0")
            P1 = None
            if len(rows) > 2:
                P1 = pspool.tile([128, 512], F32, tag="P1", bufs=2,
                                 name="P1")
            PF = None
            if any(need_foreign(s) for (s, _, _) in rows):
                PF = pspool.tile([128, 512], F32, tag="P1", bufs=2,
                                 name="PF")
            seqs = []
            folds = []
            for (s_out, ph, pidx) in rows:
                Pt = P0 if pidx == 0 else P1
                ops, f = row_taps(s_out, ph, Pt, PF)
                seqs.append(ops)
                if f:
                    folds.append((s_out, ph))
            nmax = max((len(s) for s in seqs), default=0)
            for t in range(nmax):
                for s in seqs:
                    if t < len(s):
                        nc.tensor.matmul(
                            s[t]["out"], s[t]["lhsT"], s[t]["rhs"],
                            start=s[t]["start"], stop=s[t]["stop"],
                            skip_group_check=True,
                            tile_position=s[t]["tile_position"])
            # ---- copies: psum -> xs (the whole update) ----
            if kind in ("g4", "g2p"):
                # x' = psum + C, full width (P0 halves align with xs)
                nc.vector.tensor_add(out=xs[:, ja, 1:513], in0=P0[:, :],
                                     in1=ch[:, ja, 1:513])
                if kind == "g4":
                    nc.vector.tensor_add(out=xs[TOP, jb, 1:513],
                                         in0=P1[BOT, :],
                                         in1=ch[TOP, jb, 1:513])
                    nc.vector.tensor_add(out=xs[BOT, jb, 1:513],
                                         in0=P1[TOP, :],
                                         in1=ch[BOT, jb, 1:513])
            else:
                for (s_out, ph, pidx) in rows:
                    Pt = P0 if pidx == 0 else P1
                    dhalf = _half(s_out, pi)
                    dj = _j(s_out)
                    nc.vector.tensor_add(out=xs[_P(dhalf), dj, 1:513],
                                         in0=Pt[_P(ph), :],
                                         in1=ch[_P(dhalf), dj, 1:513])
            for (s_out, ph) in folds:
                fadd(s_out, ph, PF)
            if gi == carry_gi:
                s_src = HS + T - k - 1
                j_src = s_src - RH
                dj0 = 2 if k == 1 else _j(DSLOT[k][0])
                nc.vector.tensor_copy(out=xs_nxt[BOT, dj0:dj0 + 2, :],
                                      in_=xs[BOT, j_src:j_src + 2, :])
            if k < T:
                for (j0, m, hs_) in truns[k + 1]:
                    if run_gate[(k + 1, j0)] == gi:
                        emit_tanh_run(k + 1, j0, m, hs_)
        if k == 1 and early_fn is not None:
            # emit the next tile's load-dependent tanh now so it overlaps
            # the rest of this tile instead of stalling at the boundary
            early_fn()

    # ---------------- store x_T ----------------
    s_lo_st = 4 if n == 1 else 1
    s_hi_st = (SLAB - 1 - b0 + T) if n == NT else HS
    r_top0 = b0 - T + s_lo_st
    n_top = RH - s_lo_st
    r_bot0 = b0 - T + RH
    n_bot = s_hi_st - RH + 1
    dst = x_out if last else dst_d
    nc.sync.dma_start(out=dst[:, r_top0:r_top0 + n_top, :],
                      in_=xs[_P(pi), s_lo_st:RH, :])
    nc.sync.dma_start(out=dst[:, r_bot0:r_bot0 + n_bot, :],
                      in_=xs[_P(1 - pi), 0:n_bot, :])


def host_prep(u, W_B, W_A, bias, alpha_logit):
    alpha = np.float32(1.0 / (1.0 + np.exp(-np.float64(alpha_logit))))
    beta = np.float32(1.0) - alpha

    WAe = np.array(W_A, dtype=np.float32).copy()
    idx = np.arange(64)
    WAe[idx, idx, 1, 1] = np.maximum(WAe[idx, idx, 1, 1], np.float32(1.0))

    # alpha dither: the two bf16 neighbours of alpha
    cands = np.unique(np.array(
        [ml_dtypes.bfloat16(alpha * (1 + eps))
         for eps in np.linspace(-0.02, 0.02, 2001)],
        dtype=ml_dtypes.bfloat16).astype(np.float32))
    lo_c = cands[cands <= alpha]
    hi_c = cands[cands >= alpha]
    a_lo = lo_c[-1] if len(lo_c) else np.float32(a_bf)
    a_hi = hi_c[0] if len(hi_c) else np.float32(a_bf)

    wa_taps = np.zeros((64, 12, 64), dtype=np.float32)
    for t9 in range(9):
        kh, kw = divmod(t9, 3)
        wa_taps[:, t9, :] = (beta * WAe[:, :, kh, kw]).T   # [cin, cout]
    eye = np.eye(64, dtype=np.float32)
    wa_taps[:, 9, :] = eye
    wa_taps[:, 10, :] = a_lo * eye
    wa_taps[:, 11, :] = a_hi * eye
    wa_taps = wa_taps.astype(ml_dtypes.bfloat16)

    bias_vec = np.array(bias, dtype=np.float32).reshape(64)
    wb10 = np.zeros((10, 64), dtype=np.float32)
    wb10[0, :] = bias_vec
    for t9 in range(9):
        kh, kw = divmod(t9, 3)
        wb10[t9 + 1, :] = W_B[:, 0, kh, kw]
    wb10 = wb10.astype(ml_dtypes.bfloat16)
    nbias = (-bias_vec).reshape(64, 1).astype(np.float32)
    alpha_arr = np.full((1, 1), alpha, dtype=np.float32)

    H = u.shape[2]
    in_maps = []
    for core in range(8):
        b, h = divmod(core, 2)
        img = np.asarray(u[b, 0], dtype=np.float32)        # [H, 512]
        u_slab = np.zeros((UROWS, W), dtype=np.float32)
        if h == 0:
            u_slab[1:UROWS] = img[0:SLAB + 1]
        else:
            off = H - SLAB
            u_slab[0:UROWS - 1] = img[off - 1:H]
        in_maps.append({
            "u_in": u_slab.astype(ml_dtypes.bfloat16),
            "wa_in": wa_taps,
            "wb_in": wb10,
            "nbias_in": nbias,
            "alpha_in": alpha_arr,
        })
    return in_maps


_NC_CACHE = {}


def _get_nc():
    if "nc" not in _NC_CACHE:
        _NC_CACHE["nc"] = build()
    return _NC_CACHE["nc"]


def kernel(u, W_B, W_A, bias, alpha_logit, _trace=False):
    u = np.asarray(u, dtype=np.float32)
    B, _, H, Wc = u.shape
    nc = _get_nc()
    in_maps = host_prep(u, W_B, W_A, bias, alpha_logit)
    res = run_bass_kernel_spmd(nc, in_maps, core_ids=list(range(8)),
                               trace=_trace)
    VALID = H // 2
    out = np.zeros((B, 64, H, Wc), dtype=np.float32)
    for core in range(8):
        b, h = divmod(core, 2)
        xo = np.asarray(res.results[core]["x_out"])[:, :, 1:513]
        xo = xo.astype(np.float32)
        if h == 0:
            out[b, :, 0:VALID, :] = xo[:, 0:VALID, :]
        else:
            out[b, :, VALID:H, :] = xo[:, SLAB - VALID:SLAB, :]
    kernel._last_results = res
    return out


# revision 31
# speedup vs baseline: 1.0802x; 1.0063x over previous
"""CeNN front-end Trainium2 kernel — time-skewed schedule, PE-centric update.

Reference computation (per batch image u [1,H,W]):
    control = conv3x3_same(u, W_B)                         # [64,H,W]
    x0 = control
    x_{k+1} = alpha*x_k + beta*(conv3x3_same(tanh(x_k), WA_eff) + control
                                + bias)      (WA_eff diag center >= 1), 16x.

Distribution: 8 cores = (batch b 0..3) x (H half); each core owns a
272-row slab (256 valid + 16 halo rows), zero inter-core communication.

Schedule: T=4 steps per DRAM pass (4 passes), slab processed as NT=8
sequential 34-row tiles, TIME-SKEWED (tile n at step k updates rows
(b_{n-1}-k, b_n-k]); 2-row x_k boundaries carried tile-to-tile in SBUF,
so zero halo recompute/reload.  Pass 0 (control from u) is interleaved
into pass 1.

Per-row step = 11 accumulating quadrant matmuls (K=64,M=64,N=512):
9 conv taps + identity tap adding C = beta*(control+bias) + identity tap
adding alpha*x (alpha dithered between its two bf16 neighbours per step
to fix the bf16 quantization of alpha — validated 6.5e-3 rel).  The
"update" is then just a psum->SBUF bf16 copy: full-width aligned pairs
on ScalarE, partition-crossed halves on VectorE.  tanh runs on ScalarE
merged 3 rows per op ((N+352)/1.2ns cost model makes merging ~40%
cheaper); next-step tanh is emitted as soon as its source rows' copies
are emitted, so PE never waits a step boundary.
"""

import numpy as np
import ml_dtypes

import concourse.bacc as bacc
import concourse.tile as tile
from concourse import mybir
from concourse.bass_utils import run_bass_kernel_spmd

F32 = mybir.dt.float32
BF16 = mybir.dt.bfloat16
AF = mybir.ActivationFunctionType
ALU = mybir.AluOpType

SLAB, W, WP = 272, 512, 514
T, HS = 4, 34
NT = SLAB // HS
RH = 20
NJ_TOP = 25            # j: 0..19 main, 20 CS0, 21/22 D2, 23/24 D3
CS0 = 40
DSLOT = {2: (41, 42), 3: (43, 44)}
NPASS = 16 // T
RC0 = 8
NCHUNK0 = SLAB // RC0
UROWS = SLAB + 2


def _half(s, pi):
    if s >= 40:
        return pi
    return pi if s < RH else 1 - pi


def _j(s):
    if s >= 40:
        return s - 20
    return s if s < RH else s - RH


def _P(h):
    return slice(64 * h, 64 * h + 64)


def _ssrc(n, k, d):
    """Slot holding x_{k-1}[b0+d] when tile n runs step k (d >= -k)."""
    if n == 1:
        return d + T
    if k == 1:
        return CS0 if d == -1 else d + T
    if k == 2:
        return d + T
    if d == -k:
        return DSLOT[k - 1][0]
    if d == -(k - 1):
        return DSLOT[k - 1][1]
    return d + T


def _tile_geom(n, k):
    b0 = (n - 1) * HS
    r_lo = 0 if n == 1 else b0 - k + 1
    r_hi = SLAB - 1 if n == NT else b0 + HS - k
    return b0, r_lo, r_hi, r_lo - b0 + T, r_hi - b0 + T


def _plan_step(n, k):
    """Groups for tile n step k.  Each group is (kind, rows) where rows is
    a list of (s_out, ph, pidx) with pidx 0 -> P0 / 1 -> P1 of the group.
    kinds: g4 (aligned pair + crossed pair), g2p (aligned pair), g4s
    (4 singles on 4 quadrants), g2m (leftover singles).
    Returns (groups, written: slot -> group idx, align: s_out -> bool)."""
    pi = (n - 1) % 2
    b0, r_lo, r_hi, s_lo, s_hi = _tile_geom(n, k)
    remap_j = None
    if n > 1 and k >= 3:
        remap_j = T - k + 1
    pair_js = [j for j in range(s_lo, s_hi - RH + 1) if j != remap_j]
    top_singles = [j for j in range(max(s_lo, s_hi - RH + 1), RH)]
    if remap_j is not None and remap_j >= s_lo:
        top_singles.insert(0, remap_j)
    bot_singles = [j for j in range(0, s_hi - RH + 1) if j not in pair_js]

    groups = []
    align = {}
    i = 0
    while i + 1 < len(pair_js):
        ja, jb = pair_js[i], pair_js[i + 1]
        rows = [(ja, pi, 0), (ja + RH, 1 - pi, 0),
                (jb, 1 - pi, 1), (jb + RH, pi, 1)]
        align[ja] = True
        groups.append(("g4", ja, jb, rows))
        i += 2
    if i < len(pair_js):
        ja = pair_js[i]
        rows = [(ja, pi, 0), (ja + RH, 1 - pi, 0)]
        align[ja] = True
        groups.append(("g2p", ja, None, rows))
    ti = bi = 0
    while ti < len(top_singles) or bi < len(bot_singles):
        jts = top_singles[ti:ti + 2]
        jbs = bot_singles[bi:bi + 2]
        ti += 2
        bi += 2
        jts = jts[:1]
        jbs = jbs[:1]
        ti -= 1
        bi -= 1
        rows = []
        if len(jts) >= 1:
            rows.append((jts[0], pi, 0))
        if len(jbs) >= 1:
            rows.append((jbs[0] + RH, 1 - pi, 0))
        kind = "g2m"
        groups.append((kind, None, None, rows))

    written = {}
    for gi, g in enumerate(groups):
        for (s_out, ph, pidx) in g[3]:
            written[s_out] = gi
    return groups, written, align


def _need_tb(n, k, pi):
    need = {}
    b0, r_lo, r_hi, _, _ = _tile_geom(n, k)
    for r in range(r_lo - 1, r_hi + 2):
        s = _ssrc(n, k, r - b0)
        need.setdefault(_j(s), set()).add(_half(s, pi))
    return need


def _tanh_runs(n, k, pi):
    """Merge the step's tanh coverage into runs of <=3 consecutive j with
    identical half-sets.  Returns list of (j0, m, halves)."""
    need = _need_tb(n, k, pi)
    runs = []
    for j in sorted(need):
        hs_ = need[j]
        mcap = 2 if j >= RH else 3
        if (runs and runs[-1][2] == hs_
                and (j >= RH) == (runs[-1][0] >= RH)
                and runs[-1][0] + runs[-1][1] == j
                and runs[-1][1] < mcap):
            runs[-1] = (runs[-1][0], runs[-1][1] + 1, hs_)
        else:
            runs.append((j, 1, hs_))
    return runs


class _TilePlan:
    """Per-tile tanh planning/emission state, so the previous tile's step
    loop can emit this tile's load-dependent tanh runs early."""

    def __init__(self, nc, n, xs, thpool):
        self.nc = nc
        self.n = n
        self.xs = xs
        self.thpool = thpool
        self.pi = (n - 1) % 2
        self.plans = {k: _plan_step(n, k) for k in range(1, T + 1)}
        self.truns = {k: _tanh_runs(n, k, self.pi) for k in range(1, T + 1)}
        self.thd = {k: {} for k in range(1, T + 1)}
        self.emitted = set()
        self.run_gate = {}
        for k in range(1, T + 1):
            w_prev = self.plans[k - 1][1] if k > 1 else {}
            for (j0, m, hs_) in self.truns[k]:
                g = -1
                for jj in range(j0, j0 + m):
                    for h in hs_:
                        if jj >= RH:
                            sl = jj + 20
                        else:
                            sl = jj if h == self.pi else jj + RH
                        if sl in w_prev:
                            g = max(g, w_prev[sl])
                self.run_gate[(k, j0)] = g

    def emit_run(self, k, j0, m, hs_):
        if (k, j0) in self.emitted:
            return
        self.emitted.add((k, j0))
        nc, xs = self.nc, self.xs
        if j0 >= RH:
            tt = self.thpool.tile([128, 2, WP], BF16, name="tth",
                                  tag="hold", bufs=6)
        else:
            tt = self.thpool.tile([128, 3, WP], BF16, name="tt", bufs=20)
        for jj in range(j0, j0 + m):
            self.thd[k][jj] = (tt, jj - j0)
        if hs_ == {0, 1}:
            nc.scalar.activation(out=tt[:, 0:m, :],
                                 in_=xs[:, j0:j0 + m, :], func=AF.Tanh)
        else:
            (h,) = hs_
            nc.scalar.activation(out=tt[_P(h), 0:m, :],
                                 in_=xs[_P(h), j0:j0 + m, :], func=AF.Tanh)

    def early(self):
        # load-dependent runs: all of step 1 + step-2 runs with no
        # same-tile writer (carry-fed main slots, written by prev tile)
        for (j0, m, hs_) in self.truns[1]:
            self.emit_run(1, j0, m, hs_)
        for (j0, m, hs_) in self.truns[2]:
            if self.run_gate[(2, j0)] < 0 and j0 < RH:
                self.emit_run(2, j0, m, hs_)

    def start_rest(self):
        for k in range(1, T + 1):
            for (j0, m, hs_) in self.truns[k]:
                if self.run_gate[(k, j0)] < 0:
                    self.emit_run(k, j0, m, hs_)


def build():
    nc = bacc.Bacc("TRN2", target_bir_lowering=False, debug=False,
                   num_devices=8)

    u_in = nc.dram_tensor("u_in", [UROWS, W], BF16, kind="ExternalInput")
    wa_in = nc.dram_tensor("wa_in", [64, 12, 64], BF16, kind="ExternalInput")
    wb_in = nc.dram_tensor("wb_in", [10, 64], BF16, kind="ExternalInput")
    nbias_in = nc.dram_tensor("nbias_in", [64, 1], F32, kind="ExternalInput")
    alpha_in = nc.dram_tensor("alpha_in", [1, 1], F32, kind="ExternalInput")
    x_out = nc.dram_tensor("x_out", [64, SLAB, WP], BF16,
                           kind="ExternalOutput")

    Xd = [nc.dram_tensor(f"Xd{i}", [64, SLAB, WP], BF16, kind="Internal")
          for i in range(2)]
    Chi_d = nc.dram_tensor("Chi", [64, SLAB, WP], BF16, kind="Internal")

    with tile.TileContext(nc) as tc:
        with tc.tile_pool(name="singles", bufs=1) as singles:
            wa_t = singles.tile([128, 12, 64], BF16)
            nc.sync.dma_start(out=wa_t[0:64], in_=wa_in[:, :, :])
            nc.sync.dma_start(out=wa_t[64:128], in_=wa_in[:, :, :])
            wb_t = singles.tile([10, 64], BF16)
            nc.sync.dma_start(out=wb_t, in_=wb_in[:, :])
            nbias_t = singles.tile([64, 1], F32)
            nc.sync.dma_start(out=nbias_t, in_=nbias_in[:, :])
            alpha_t = singles.tile([128, 1], F32)
            nc.sync.dma_start(out=alpha_t,
                              in_=alpha_in[:, :].to_broadcast((128, 1)))
            beta_t = singles.tile([128, 1], F32)
            nc.vector.tensor_scalar(out=beta_t, in0=alpha_t, scalar1=-1.0,
                                    scalar2=1.0, op0=ALU.mult, op1=ALU.add)

            with tc.tile_pool(name="p0u", bufs=2) as p0u, \
                 tc.tile_pool(name="p0st", bufs=1) as p0st, \
                 tc.tile_pool(name="xs", bufs=2) as xpool, \
                 tc.tile_pool(name="chs", bufs=2) as chpool, \
                 tc.tile_pool(name="th", bufs=16) as thpool, \
                 tc.tile_pool(name="ps", bufs=3, space="PSUM") as pspool:

                u9bufs = [p0u.tile([10, RC0, W], BF16, tag="u9",
                                   name="u9a"),
                          p0u.tile([10, RC0, W], BF16, tag="u9",
                                   name="u9b")]
                for _u9 in u9bufs:
                    nc.vector.memset(_u9[0:1, :, :], 1.0)

                def emit_chunk(c):
                    c0 = RC0 * c
                    u9 = u9bufs[c % 2]
                    # zero edge cols (kw=0 misses col 0, kw=2 misses col
                    # 511), then restore the ones-row edges
                    nc.vector.memset(u9[0:10, 0:RC0, 0:1], 0.0)
                    nc.vector.memset(u9[0:10, 0:RC0, W - 1:W], 0.0)
                    nc.vector.memset(u9[0:1, 0:RC0, 0:1], 1.0)
                    nc.vector.memset(u9[0:1, 0:RC0, W - 1:W], 1.0)
                    for t9 in range(9):
                        kh, kw = divmod(t9, 3)
                        c_lo = max(0, 1 - kw)
                        c_hi = min(W, W + 1 - kw)
                        nc.gpsimd.dma_start(
                            out=u9[t9 + 1:t9 + 2, 0:RC0, c_lo:c_hi],
                            in_=u_in[c0 + kh:c0 + kh + RC0,
                                     c_lo + kw - 1:c_hi + kw - 1],
                        )
                    xst = p0st.tile([64, RC0, WP], BF16, tag="xst",
                                    name="xst")
                    chst = p0st.tile([64, RC0, WP], BF16, tag="chst",
                                     name="chst")
                    for st in (xst, chst):
                        nc.vector.memset(st[:, :, 0:1], 0.0)
                        nc.vector.memset(st[:, :, 513:514], 0.0)
                    for t in range(RC0):
                        pc = pspool.tile([64, 512], F32, tag="pc", bufs=2,
                                         name="pc")
                        nc.tensor.matmul(pc, wb_t[0:10, :], u9[0:10, t, :],
                                         start=True, stop=True)
                        nc.scalar.activation(out=xst[:, t, 1:513], in_=pc,
                                             func=AF.Identity,
                                             bias=nbias_t[0:64], scale=1.0)
                        nc.vector.tensor_scalar(
                            out=chst[:, t, 1:513], in0=pc,
                            scalar1=beta_t[0:64], scalar2=None, op0=ALU.mult)
                    nc.sync.dma_start(out=Xd[0][:, c0:c0 + RC0, :], in_=xst)
                    nc.sync.dma_start(out=Chi_d[:, c0:c0 + RC0, :], in_=chst)

                chunks_done = 0
                tiles = [(p, n) for p in range(1, NPASS + 1)
                         for n in range(1, NT + 1)]
                bufs = {}

                tps = {}

                def alloc(i):
                    bufs[i] = (xpool.tile([128, NJ_TOP, WP], BF16,
                                          tag="xs", name="xs"),
                               chpool.tile([128, RH, WP], BF16, tag="ch",
                                           name="ch"))

                def loads(i):
                    pp, nn = tiles[i]
                    xsb, chb = bufs[i]
                    _emit_loads(nc, nn, xsb, chb, Xd[(pp - 1) % 2], Chi_d)

                alloc(0)
                for i, (p, n) in enumerate(tiles):
                    if p == 1:
                        need_c = min(NCHUNK0, -(-(n * HS + 36) // RC0))
                        if n == NT:
                            need_c = NCHUNK0
                        while chunks_done < need_c:
                            emit_chunk(chunks_done)
                            chunks_done += 1
                    if i == 0:
                        loads(0)
                        tps[0] = _TilePlan(nc, n, bufs[0][0], thpool)
                        tps[0].early()
                    if i + 1 < len(tiles):
                        alloc(i + 1)
                        loads(i + 1)
                        tps[i + 1] = _TilePlan(nc, tiles[i + 1][1],
                                               bufs[i + 1][0], thpool)
                    xs_cur, ch_cur = bufs.pop(i)
                    xs_nxt = bufs[i + 1][0] if (n < NT and i + 1 in bufs) \
                        else None
                    tp = tps.pop(i)
                    tp.start_rest()
                    early_fn = tps[i + 1].early if i + 1 in tps else None
                    _emit_tile(nc, p, n, tp, xs_cur, xs_nxt, ch_cur,
                               Xd[(p - 1) % 2], Xd[p % 2], Chi_d, x_out,
                               wa_t, thpool, pspool, p == NPASS, early_fn)

    nc.compile()
    return nc


def _emit_loads(nc, n, xs, ch, src_d, Chi_d):
    pi = (n - 1) % 2
    TOP, BOT = _P(pi), _P(1 - pi)
    b0 = (n - 1) * HS
    if n == 1:
        nc.vector.memset(xs[TOP, 2:4, :], 0.0)
        nc.vector.memset(xs[TOP, 20:21, :], 0.0)
    else:
        nc.sync.dma_start(out=xs[TOP, 20:21, :],
                          in_=src_d[:, b0 - 1:b0, :])
        nc.vector.memset(xs[TOP, 1:2, :], 0.0)
    nc.sync.dma_start(out=xs[TOP, 4:20, :], in_=src_d[:, b0:b0 + 16, :])
    if n == NT:
        nc.sync.dma_start(out=xs[BOT, 0:18, :],
                          in_=src_d[:, b0 + 16:b0 + 34, :])
        nc.vector.memset(xs[BOT, 18:19, :], 0.0)
    else:
        nc.sync.dma_start(out=xs[BOT, 0:19, :],
                          in_=src_d[:, b0 + 16:b0 + 35, :])
    if n == 1:
        nc.gpsimd.dma_start(out=ch[TOP, 4:20, :], in_=Chi_d[:, 0:16, :])
    else:
        nc.gpsimd.dma_start(out=ch[TOP, 1:20, :],
                            in_=Chi_d[:, b0 - 3:b0 + 16, :])
    nc.gpsimd.dma_start(out=ch[BOT, 0:18, :],
                        in_=Chi_d[:, b0 + 16:b0 + 34, :])


def _emit_tile(nc, p, n, tp, xs, xs_nxt, ch, src_d, dst_d, Chi_d, x_out,
               wa_t, thpool, pspool, last, early_fn):
    pi = (n - 1) % 2
    TOP, BOT = _P(pi), _P(1 - pi)
    b0 = (n - 1) * HS
    plans = tp.plans
    truns = tp.truns
    thd = tp.thd
    run_gate = tp.run_gate
    emit_tanh_run = tp.emit_run

    # ---------------- steps ----------------
    for k in range(1, T + 1):
        groups, written, align = plans[k]
        th = thd[k]
        atap = 10 + (((p - 1) * T + (k - 1)) % 2)

        def ssrc(d):
            return _ssrc(n, k, d)

        def row_taps(s_out, ph, ps_tile, pf_tile):
            d = s_out - T
            dhalf = _half(s_out, pi)
            dj = _j(s_out)
            out_ps = ps_tile[ph * 64:ph * 64 + 64, :]
            main, foreign = [], []
            for t9 in range(9):
                kh, kw = divmod(t9, 3)
                ss = ssrc(d + kh - 1)
                shalf, sj = _half(ss, pi), _j(ss)
                ent = (t9, shalf, sj, kw)
                (main if shalf == dhalf else foreign).append(ent)
            ops = []
            for idx, (t9, shalf, sj, kw) in enumerate(main):
                tt, off = th[sj]
                ops.append(dict(
                    out=out_ps, lhsT=wa_t[_P(shalf), t9, :],
                    rhs=tt[_P(shalf), off, kw:kw + 512],
                    start=(idx == 0), stop=False,
                    tile_position=(shalf * 64, ph * 64)))
            # alpha tap: rhs is raw x_{k-1} of this row (dithered weight)
            sin = ssrc(d)
            ops.append(dict(
                out=out_ps, lhsT=wa_t[_P(dhalf), atap, :],
                rhs=xs[_P(dhalf), _j(sin), 1:513],
                start=False, stop=True,
                tile_position=(dhalf * 64, ph * 64)))
            if foreign:
                out_pf = pf_tile[ph * 64:ph * 64 + 64, :]
                for idx, (t9, shalf, sj, kw) in enumerate(foreign):
                    tt, off = th[sj]
                    ops.append(dict(
                        out=out_pf, lhsT=wa_t[_P(shalf), t9, :],
                        rhs=tt[_P(shalf), off, kw:kw + 512],
                        start=(idx == 0), stop=(idx == len(foreign) - 1),
                        tile_position=(shalf * 64, ph * 64)))
            return ops, bool(foreign)

        def fadd(s_out, ph, pf_tile):
            dhalf = _half(s_out, pi)
            dj = _j(s_out)
            nc.vector.scalar_tensor_tensor(
                out=xs[_P(dhalf), dj, 1:513],
                in0=xs[_P(dhalf), dj, 1:513],
                scalar=1.0, in1=pf_tile[_P(ph), :],
                op0=ALU.bypass, op1=ALU.add)

        def need_foreign(s_out):
            d = s_out - T
            dhalf = _half(s_out, pi)
            for kh in (0, 2):
                if _half(ssrc(d + kh - 1), pi) != dhalf:
                    return True
            return False

        carry_gi = -1
        if k < T and xs_nxt is not None:
            carry_gi = max(written[HS + T - k - 1], written[HS + T - k])

        for gi, g in enumerate(groups):
            kind, ja, jb, rows = g
            P0 = pspool.tile([128, 512], F32, tag="P0", bufs=4, name="P0")
            P1 = None
            if len(rows) > 2:
                P1 = pspool.tile([128, 512], F32, tag="P1", bufs=2,
                                 name="P1")
            PF = None
            if any(need_foreign(s) for (s, _, _) in rows):
                PF = pspool.tile([128, 512], F32, tag="P1", bufs=2,
                                 name="PF")
            seqs = []
            folds = []
            for (s_out, ph, pidx) in rows:
                Pt = P0 if pidx == 0 else P1
                ops, f = row_taps(s_out, ph, Pt, PF)
                seqs.append(ops)
                if f:
                    folds.append((s_out, ph))
            nmax = max((len(s) for s in seqs), default=0)
            for t in range(nmax):
                for s in seqs:
                    if t < len(s):
                        nc.tensor.matmul(
                            s[t]["out"], s[t]["lhsT"], s[t]["rhs"],
                            start=s[t]["start"], stop=s[t]["stop"],
                            skip_group_check=True,
                            tile_position=s[t]["tile_position"])
            # ---- copies: psum -> xs (the whole update) ----
            if kind in ("g4", "g2p"):
                # x' = psum + C, full width (P0 halves align with xs)
                nc.vector.tensor_add(out=xs[:, ja, 1:513], in0=P0[:, :],
                                     in1=ch[:, ja, 1:513])
                if kind == "g4":
                    nc.vector.tensor_add(out=xs[TOP, jb, 1:513],
                                         in0=P1[BOT, :],
                                         in1=ch[TOP, jb, 1:513])
                    nc.vector.tensor_add(out=xs[BOT, jb, 1:513],
                                         in0=P1[TOP, :],
                                         in1=ch[BOT, jb, 1:513])
            else:
                for (s_out, ph, pidx) in rows:
                    Pt = P0 if pidx == 0 else P1
                    dhalf = _half(s_out, pi)
                    dj = _j(s_out)
                    nc.vector.tensor_add(out=xs[_P(dhalf), dj, 1:513],
                                         in0=Pt[_P(ph), :],
                                         in1=ch[_P(dhalf), dj, 1:513])
            for (s_out, ph) in folds:
                fadd(s_out, ph, PF)
            if gi == carry_gi:
                s_src = HS + T - k - 1
                j_src = s_src - RH
                dj0 = 2 if k == 1 else _j(DSLOT[k][0])
                nc.vector.tensor_copy(out=xs_nxt[BOT, dj0:dj0 + 2, :],
                                      in_=xs[BOT, j_src:j_src + 2, :])
            if k < T:
                for (j0, m, hs_) in truns[k + 1]:
                    if run_gate[(k + 1, j0)] == gi:
                        emit_tanh_run(k + 1, j0, m, hs_)
        if k == 1 and early_fn is not None:
            # emit the next tile's load-dependent tanh now so it overlaps
            # the rest of this tile instead of stalling at the boundary
            early_fn()

    # ---------------- store x_T ----------------
    s_lo_st = 4 if n == 1 else 1
    s_hi_st = (SLAB - 1 - b0 + T) if n == NT else HS
    r_top0 = b0 - T + s_lo_st
    n_top = RH - s_lo_st
    r_bot0 = b0 - T + RH
    n_bot = s_hi_st - RH + 1
    dst = x_out if last else dst_d
    nc.sync.dma_start(out=dst[:, r_top0:r_top0 + n_top, :],
                      in_=xs[_P(pi), s_lo_st:RH, :])
    nc.sync.dma_start(out=dst[:, r_bot0:r_bot0 + n_bot, :],
                      in_=xs[_P(1 - pi), 0:n_bot, :])


def host_prep(u, W_B, W_A, bias, alpha_logit):
    alpha = np.float32(1.0 / (1.0 + np.exp(-np.float64(alpha_logit))))
    beta = np.float32(1.0) - alpha

    WAe = np.array(W_A, dtype=np.float32).copy()
    idx = np.arange(64)
    WAe[idx, idx, 1, 1] = np.maximum(WAe[idx, idx, 1, 1], np.float32(1.0))

    # alpha dither: the two bf16 neighbours of alpha
    cands = np.unique(np.array(
        [ml_dtypes.bfloat16(alpha * (1 + eps))
         for eps in np.linspace(-0.02, 0.02, 2001)],
        dtype=ml_dtypes.bfloat16).astype(np.float32))
    lo_c = cands[cands <= alpha]
    hi_c = cands[cands >= alpha]
    a_lo = lo_c[-1] if len(lo_c) else np.float32(a_bf)
    a_hi = hi_c[0] if len(hi_c) else np.float32(a_bf)

    wa_taps = np.zeros((64, 12, 64), dtype=np.float32)
    for t9 in range(9):
        kh, kw = divmod(t9, 3)
        wa_taps[:, t9, :] = (beta * WAe[:, :, kh, kw]).T   # [cin, cout]
    eye = np.eye(64, dtype=np.float32)
    wa_taps[:, 9, :] = eye
    wa_taps[:, 10, :] = a_lo * eye
    wa_taps[:, 11, :] = a_hi * eye
    wa_taps = wa_taps.astype(ml_dtypes.bfloat16)

    bias_vec = np.array(bias, dtype=np.float32).reshape(64)
    wb10 = np.zeros((10, 64), dtype=np.float32)
    wb10[0, :] = bias_vec
    for t9 in range(9):
        kh, kw = divmod(t9, 3)
        wb10[t9 + 1, :] = W_B[:, 0, kh, kw]
    wb10 = wb10.astype(ml_dtypes.bfloat16)
    nbias = (-bias_vec).reshape(64, 1).astype(np.float32)
    alpha_arr = np.full((1, 1), alpha, dtype=np.float32)

    H = u.shape[2]
    in_maps = []
    for core in range(8):
        b, h = divmod(core, 2)
        img = np.asarray(u[b, 0], dtype=np.float32)        # [H, 512]
        u_slab = np.zeros((UROWS, W), dtype=np.float32)
        if h == 0:
            u_slab[1:UROWS] = img[0:SLAB + 1]
        else:
            off = H - SLAB
            u_slab[0:UROWS - 1] = img[off - 1:H]
        in_maps.append({
            "u_in": u_slab.astype(ml_dtypes.bfloat16),
            "wa_in": wa_taps,
            "wb_in": wb10,
            "nbias_in": nbias,
            "alpha_in": alpha_arr,
        })
    return in_maps


_NC_CACHE = {}


def _get_nc():
    if "nc" not in _NC_CACHE:
        _NC_CACHE["nc"] = build()
    return _NC_CACHE["nc"]


def kernel(u, W_B, W_A, bias, alpha_logit, _trace=False):
    u = np.asarray(u, dtype=np.float32)
    B, _, H, Wc = u.shape
    nc = _get_nc()
    in_maps = host_prep(u, W_B, W_A, bias, alpha_logit)
    res = run_bass_kernel_spmd(nc, in_maps, core_ids=list(range(8)),
                               trace=_trace)
    VALID = H // 2
    out = np.zeros((B, 64, H, Wc), dtype=np.float32)
    for core in range(8):
        b, h = divmod(core, 2)
        xo = np.asarray(res.results[core]["x_out"])[:, :, 1:513]
        xo = xo.astype(np.float32)
        if h == 0:
            out[b, :, 0:VALID, :] = xo[:, 0:VALID, :]
        else:
            out[b, :, VALID:H, :] = xo[:, SLAB - VALID:SLAB, :]
    kernel._last_results = res
    return out
